# revision 1
# baseline (speedup 1.0000x reference)
"""GAT (2-layer, PyG-style) on 8 Trainium2 NeuronCores — v2.

Design vs baseline:
  - Layer-1 dense phase is REPLICATED on every core (PE is cheap), writing a
    local bf16 table [R, 384] = [h 256 | as 8 | pad], so the 64MB table-1
    AllGather disappears entirely.
  - All gather tables and PE matmuls in bf16 (2x PE rate, ~2x less HBM).
  - One dma_gather per (dst-block, half) instead of 6-tile chunks (amortizes
    the ~1us SWDGE fixed cost); descriptor ring enlarged.
  - One-hot "St" (dst-major) built via rank-1 PE broadcast of host-supplied
    dlocT + a single is_equal per 512-col chunk (no per-tile PE transposes).
  - X is fed host-pretransposed+interleaved so the dense phases do straight
    matmuls with no on-device transposes.
  - Node rows are laid out core-major (r = core*NPAD + local) for BOTH
    tables, so one int16 index set serves both layers.
"""
import sys
sys.path.insert(0, "/opt/trn_rl_repo")

import numpy as np
import concourse.bass as bass
import concourse.bacc as bacc
import concourse.mybir as mybir
from concourse.tile import TileContext
from concourse.bass_utils import run_bass_kernel_spmd

F32 = mybir.dt.float32
BF16 = mybir.dt.bfloat16
I16 = mybir.dt.int16

P = 128
NCORES = 8
LEAKY = 0.2
EPS = 1e-16


class Cfg:
    def __init__(self, N, E, IN_CH=256, HID=256, OUT_CH=64, H1=8):
        self.N, self.E = N, E
        self.IN_CH, self.HID, self.OUT_CH, self.H1 = IN_CH, HID, OUT_CH, H1
        self.C1 = HID // H1
        self.ND = N // NCORES                    # dst nodes per core
        self.NB = (self.ND + P - 1) // P         # dst blocks per core
        self.NPAD = self.NB * P                  # padded shard rows
        self.R = NCORES * self.NPAD              # global table rows
        self.RB = NCORES * self.NB               # global table blocks
        # asymmetric int16 split: smaller lo half -> fewer gather chunks
        # per block (TA~7 fits one 7-tile call) and lo table ready earlier
        self.HALF = 140 * P if self.RB == 392 else self.R // 2
        assert self.HALF % P == 0
        assert self.HALF < 32768 and (self.R - self.HALF) < 32768
        # layer-2 table split by shard-row halves so each half can be
        # AllGathered separately (first half fires mid-phase-B)
        self.NB_LO = 18 if self.NB == 49 else (self.NB + 1) // 2
        self.SL = self.NB_LO * P                 # shard rows in half 0
        self.HALF2 = NCORES * self.SL            # table2 half boundary
        assert self.HALF2 < 32768 and (self.R - self.HALF2) < 32768
        self.TW1 = 384                           # table1 row width bf16 (264 used)
        self.U1 = HID + H1                       # 264
        self.TW2 = 128                           # table2 row width bf16 (65 used)
        self.U2 = OUT_CH + 1                     # 65 = [g 64 | as2]


CFG_FULL = Cfg(N=50000, E=800000)


# ---------------------------------------------------------------- host side
def _node_row(cfg, n):
    # core-major padded row layout, same for table1 and table2
    return (n // cfg.ND) * cfg.NPAD + (n % cfg.ND)


def _node_row2(cfg, n):
    # layer-2 layout: shard-half-major so each half AllGathers contiguously
    c = n // cfg.ND
    l = n % cfg.ND
    return np.where(l < cfg.SL, c * cfg.SL + l,
                    cfg.HALF2 + c * (cfg.NPAD - cfg.SL) + (l - cfg.SL))


def preprocess_graph(cfg, edge_index, row_of=_node_row, half=None):
    """Per-core wrapped int16 gather indices, dloc (edge-major) and dlocT
    (flat, for the St broadcast), plus shared per-block tile counts."""
    if half is None:
        half = cfg.HALF
    src = np.concatenate([edge_index[0], np.arange(cfg.N, dtype=np.int64)])
    dst = np.concatenate([edge_index[1], np.arange(cfg.N, dtype=np.int64)])
    r_src = row_of(cfg, src)

    core = dst // cfg.ND
    dst_local = dst - core * cfg.ND
    blk = dst_local // P
    dloc = dst_local % P
    in_a = r_src < half

    NB = cfg.NB
    counts = np.zeros((NCORES, NB, 2), dtype=np.int64)
    np.add.at(counts, (core, blk, (~in_a).astype(np.int64)), 1)
    TA = np.maximum(1, (counts[:, :, 0].max(0) + P - 1) // P)
    TB = np.maximum(1, (counts[:, :, 1].max(0) + P - 1) // P)

    order = np.lexsort((in_a * -1, blk, core))  # by core, block, half (A first)
    rsrc_s, core_s, blk_s, dloc_s, ina_s = (
        r_src[order], core[order], blk[order], dloc[order], in_a[order])

    Tsum = int((TA + TB).sum())
    idx16 = []   # per core: [128, 8 * Tsum] int16
    dlocf = []   # per core: [128, Tsum] bf16-able float
    dloct = []   # per core: [1, Tsum*128] float
    for c in range(NCORES):
        iw = np.zeros((P, 8 * Tsum), dtype=np.int16)
        dw = np.full((P, Tsum), -1.0, dtype=np.float32)
        dt_ = np.full((1, Tsum * P), -1.0, dtype=np.float32)
        csel = core_s == c
        col0 = 0
        for b in range(NB):
            bsel = csel & (blk_s == b)
            for hh, T in ((0, int(TA[b])), (1, int(TB[b]))):
                hsel = bsel & (ina_s == (hh == 0))
                rr = rsrc_s[hsel] - (0 if hh == 0 else half)
                dd = dloc_s[hsel]
                S = T * P
                assert len(rr) <= S
                idx = np.zeros(S, dtype=np.int16)
                idx[: len(rr)] = rr.astype(np.int16)
                dl = np.full(S, -1.0, dtype=np.float32)
                dl[: len(dd)] = dd.astype(np.float32)
                w = idx.reshape(S // 16, 16).T          # [16, S/16] wrap
                iw[:, 8 * col0: 8 * col0 + S // 16] = np.tile(w, (8, 1))
                dw[:, col0: col0 + T] = dl.reshape(T, P).T
                dt_[0, col0 * P: col0 * P + S] = dl
                col0 += T
        assert col0 == Tsum
        idx16.append(iw)
        dlocf.append(dw)
        dloct.append(dt_)
    return idx16, dlocf, dloct, TA.astype(int).tolist(), TB.astype(int).tolist()


def make_weights(cfg, W1, att_src1, att_dst1, W2, att_src2, att_dst2):
    H1, C1 = cfg.H1, cfg.C1
    A1s = np.zeros((cfg.HID, H1), dtype=np.float64)
    A1s[np.arange(cfg.HID), np.arange(cfg.HID) // C1] = att_src1.ravel()
    A1d = np.zeros((cfg.HID, H1), dtype=np.float64)
    A1d[np.arange(cfg.HID), np.arange(cfg.HID) // C1] = att_dst1.ravel()
    # [h 256 | as 8 | ad 8] -> 272 cols
    W1f = np.concatenate([W1, W1 @ A1s, W1 @ A1d], axis=1).astype(np.float32)
    # [g 64 | as2 1 | ad2 1] -> 66 cols
    W2f = np.concatenate([W2, W2 @ att_src2.T, W2 @ att_dst2.T],
                         axis=1).astype(np.float32)
    return W1f, W2f


def bf16(a):
    import ml_dtypes
    return np.asarray(a, dtype=ml_dtypes.bfloat16)


# ---------------------------------------------------------------- device side
GATHER_CHUNK_TILES = 7   # 896 idxs per call; >=1280 fails on HW (ring limit)


def _gather_chunks(TA, TB, tbl_lo, tbl_hi):
    out = []
    for lo, hi, tbl in ((0, TA, tbl_lo), (TA, TA + TB, tbl_hi)):
        c = lo
        while c < hi:
            e = min(c + GATHER_CHUNK_TILES, hi)
            out.append((c, e, tbl))
            c = e
    return out


def build_kernel(cfg, TA, TB, Tsum, TA2, TB2, Tsum2):
    nc = bacc.Bacc("TRN2", target_bir_lowering=False, debug=False,
                   num_devices=NCORES)
    IN, HID, OUT, H1, C1 = cfg.IN_CH, cfg.HID, cfg.OUT_CH, cfg.H1, cfg.C1
    U1, U2, TW1, TW2 = cfg.U1, cfg.U2, cfg.TW1, cfg.TW2
    NB, NPAD, R, RB, HALF = cfg.NB, cfg.NPAD, cfg.R, cfg.RB, cfg.HALF
    KI = IN // P   # k-chunks for layer-1 dense
    KH = HID // P  # k-chunks for layer-2 dense
    W1W = U1 + H1  # 272

    # host-pretransposed, per-block interleaved X: [128, RB*KI*128] bf16
    XTI = nc.declare_dram_parameter("XTI", [P, RB * KI * P], BF16, isOutput=False)
    XSI = nc.declare_dram_parameter("XSI", [P, NB * KI * P], BF16, isOutput=False)
    W1F = nc.declare_dram_parameter("W1F", [P, KI, W1W], BF16, isOutput=False)
    W2F = nc.declare_dram_parameter("W2F", [P, KH, U2 + 1], BF16, isOutput=False)
    TDL = nc.declare_dram_parameter("TDL", [P, 9 * Tsum], I16, isOutput=False)
    DLOCT = nc.declare_dram_parameter("DLOCT", [1, Tsum * P], BF16, isOutput=False)
    TDL2 = nc.declare_dram_parameter("TDL2", [P, 9 * Tsum2], I16, isOutput=False)
    DLOCT2 = nc.declare_dram_parameter("DLOCT2", [1, Tsum2 * P], BF16,
                                       isOutput=False)
    IOTA = nc.declare_dram_parameter("IOTA", [P, P], BF16, isOutput=False)
    IOTAC = nc.declare_dram_parameter("IOTAC", [P, 1], F32, isOutput=False)
    ONES1 = nc.declare_dram_parameter("ONES1", [1, P], BF16, isOutput=False)
    IDENT = nc.declare_dram_parameter("IDENT", [P, P], F32, isOutput=False)
    B1R = nc.declare_dram_parameter("B1R", [P, HID], F32, isOutput=False)
    B2R = nc.declare_dram_parameter("B2R", [P, OUT], F32, isOutput=False)
    OUTT = nc.declare_dram_parameter("OUTT", [cfg.ND, OUT], F32, isOutput=True)

    with TileContext(nc, num_cores=NCORES) as tc:
        with (
            tc.tile_pool(name="const", bufs=1) as cpool,
            tc.tile_pool(name="dram", bufs=1, space="DRAM") as dram,
        ):
            # resident constants
            iota_sb = cpool.tile([P, P], BF16)
            nc.sync.dma_start(out=iota_sb[:], in_=IOTA[:, :])
            iotac_sb = cpool.tile([P, 1], F32)
            nc.sync.dma_start(out=iotac_sb[:], in_=IOTAC[:, :])
            ones1_sb = cpool.tile([1, P], BF16)
            nc.sync.dma_start(out=ones1_sb[:], in_=ONES1[:, :])
            ident_sb = cpool.tile([P, P], F32)
            nc.sync.dma_start(out=ident_sb[:], in_=IDENT[:, :])
            b1_sb = cpool.tile([P, HID], F32)
            nc.sync.dma_start(out=b1_sb[:], in_=B1R[:, :])
            b2_sb = cpool.tile([P, OUT], F32)
            nc.sync.dma_start(out=b2_sb[:], in_=B2R[:, :])
            w1f_sb = cpool.tile([P, KI, W1W], BF16)
            for k in range(KI):
                nc.sync.dma_start(out=w1f_sb[:, k, :], in_=W1F[:, k, :])
            w2f_sb = cpool.tile([P, KH, U2 + 1], BF16)
            for k in range(KH):
                nc.sync.dma_start(out=w2f_sb[:, k, :], in_=W2F[:, k, :])
            # resident per-own-block attention-dst logits
            ad1_sb = cpool.tile([P, NB, H1], BF16)
            ad2_sb = cpool.tile([P, NB], F32)

            table1_lo = dram.tile([HALF, TW1], BF16)
            table1_hi = dram.tile([R - HALF, TW1], BF16)
            SL, HALF2 = cfg.SL, cfg.HALF2
            shard2_lo = dram.tile([SL, TW2], BF16)
            shard2_hi = dram.tile([NPAD - SL, TW2], BF16)
            table2_lo = dram.tile([HALF2, TW2], BF16, addr_space="Shared")
            table2_hi = dram.tile([R - HALF2, TW2], BF16, addr_space="Shared")

            # ---------------- phase A: replicated layer-1 dense, local table1
            # group size divides each table half so no group straddles lo/hi
            HB_LO = HALF // P
            GA = max(g for g in range(1, 9)
                     if HB_LO % g == 0 and (RB - HB_LO) % g == 0)
            NG_LO = HB_LO // GA
            # phase-B pools open FIRST so B's gather destinations do not
            # alias phase-A tiles (aliasing would serialize B behind A)
            KLA = min(5, NB)  # half-lo gather lookahead (blocks)
            pb_scope = (
                tc.tile_pool(name="pb_heA", bufs=KLA + 2),
                tc.tile_pool(name="pb_heB", bufs=2),
                tc.tile_pool(name="pb_tdl", bufs=KLA + 2),
                tc.tile_pool(name="pb_sb", bufs=2),
                tc.tile_pool(name="pb_small", bufs=2),
                tc.tile_pool(name="pb_dlt", bufs=2),
            )
            import contextlib as _cl
            _stack = _cl.ExitStack()
            p_heA = _stack.enter_context(pb_scope[0])
            p_heB = _stack.enter_context(pb_scope[1])
            p_tdl = _stack.enter_context(pb_scope[2])
            sb_b = _stack.enter_context(pb_scope[3])
            sm_b = _stack.enter_context(pb_scope[4])
            p_dlt = _stack.enter_context(pb_scope[5])
            with (
                tc.tile_pool(name="pa_sb", bufs=2) as sb,
                tc.tile_pool(name="pa_ps", bufs=4, space="PSUM") as ps,
            ):
                for g in range(RB // GA):
                    xt = sb.tile([P, GA, KI, P], BF16, tag="xt")
                    nc.sync.dma_start(
                        out=xt[:],
                        in_=XTI[:, g * GA * KI * P:(g + 1) * GA * KI * P])
                    hrow = sb.tile([P, GA, U1], BF16, tag="hrow")
                    for s in range(GA):
                        ph = ps.tile([P, U1], F32, tag="ph")
                        for k in range(KI):
                            nc.tensor.matmul(out=ph[:], lhsT=xt[:, s, k, :],
                                             rhs=w1f_sb[:, k, 0:U1],
                                             start=(k == 0), stop=(k == KI - 1))
                        if s % 2 == 0:
                            nc.scalar.copy(out=hrow[:, s, :], in_=ph[:])
                        else:
                            nc.vector.tensor_scalar(
                                out=hrow[:, s, :], in0=ph[:], scalar1=0.0,
                                scalar2=None, op0=mybir.AluOpType.add)
                    # [p, s, c] -> DRAM rows (g*GA+s)*128+p, cols 0:U1
                    if g < NG_LO:
                        wdst = table1_lo[g * GA * P:(g + 1) * GA * P, 0:U1]
                    else:
                        g2 = g - NG_LO
                        wdst = table1_hi[g2 * GA * P:(g2 + 1) * GA * P, 0:U1]
                    nc.scalar.dma_start(
                        out=wdst.rearrange("(s p) c -> p s c", p=P),
                        in_=hrow[:])

                # phase A': own-shard attention-dst logits (tiny, resident)
                GB = max(g for g in range(1, 9) if NB % g == 0)
                for g in range(NB // GB):
                    xs = sb.tile([P, GB, KI, P], BF16, tag="xs")
                    nc.sync.dma_start(
                        out=xs[:],
                        in_=XSI[:, g * GB * KI * P:(g + 1) * GB * KI * P])
                    for s in range(GB):
                        pa = ps.tile([P, H1], F32, tag="pa")
                        for k in range(KI):
                            nc.tensor.matmul(out=pa[:], lhsT=xs[:, s, k, :],
                                             rhs=w1f_sb[:, k, U1:W1W],
                                             start=(k == 0), stop=(k == KI - 1))
                        nc.scalar.copy(out=ad1_sb[:, g * GB + s, :], in_=pa[:])

            # ---------------- phase B: layer-1 edge aggregation + L2 dense
            # (pipelined: half-lo gathers issued KLA blocks ahead so their
            #  SWDGE descriptor generation overlaps phase A's second half)
            with (
                tc.tile_pool(name="pb_ps", bufs=2, space="PSUM") as ps,
                tc.tile_pool(name="pb_ps1", bufs=1, space="PSUM") as ps1,
            ):
                sb = sb_b
                sm = sm_b
                icols = []
                _ic = 0
                for b in range(NB):
                    icols.append(_ic)
                    _ic += TA[b] + TB[b]
                tdls = [None] * NB
                heAs = [None] * NB
                for i in range(NB + KLA):
                    if i < NB:
                        # issue stage: tdl load + half-lo gathers (depend only
                        # on table1_lo, ready at phase-A midpoint)
                        b = i
                        T = TA[b] + TB[b]
                        ic = icols[b]
                        tdl = p_tdl.tile([P, 9 * T], I16, tag="tdl")
                        nc.sync.dma_start(out=tdl[:],
                                          in_=TDL[:, 9 * ic: 9 * (ic + T)])
                        tdls[b] = tdl
                        heA = p_heA.tile([P, TA[b], TW1], BF16, tag="heA")
                        for c0 in range(0, TA[b], GATHER_CHUNK_TILES):
                            c1 = min(c0 + GATHER_CHUNK_TILES, TA[b])
                            nc.gpsimd.dma_gather(
                                heA[:, c0:c1, :], table1_lo[:, :],
                                tdl[:, 8 * c0:8 * c1],
                                num_idxs=(c1 - c0) * P,
                                num_idxs_reg=(c1 - c0) * P,
                                elem_size=TW1)
                        heAs[b] = heA
                    if i < KLA:
                        continue
                    # compute stage (block b = i - KLA): half-hi gather + rest
                    b = i - KLA
                    T = TA[b] + TB[b]
                    TAb = TA[b]
                    S128 = T * P
                    ic = icols[b]
                    tdl = tdls[b]
                    heA = heAs[b]
                    tdls[b] = heAs[b] = None
                    dloc = tdl[:, 8 * T:9 * T].bitcast(BF16)

                    heB = p_heB.tile([P, TB[b], TW1], BF16, tag="heB")
                    for c0 in range(0, TB[b], GATHER_CHUNK_TILES):
                        c1 = min(c0 + GATHER_CHUNK_TILES, TB[b])
                        nc.gpsimd.dma_gather(
                            heB[:, c0:c1, :], table1_hi[:, :],
                            tdl[:, 8 * (TAb + c0):8 * (TAb + c1)],
                            num_idxs=(c1 - c0) * P,
                            num_idxs_reg=(c1 - c0) * P,
                            elem_size=TW1)

                    dlocT = p_dlt.tile([1, S128], BF16, tag="dlocT")
                    nc.scalar.dma_start(
                        out=dlocT[:], in_=DLOCT[:, ic * P: (ic + T) * P])

                    # S[e, t, d] one-hot (edge-major) for aggregation
                    S = sb.tile([P, T, P], BF16, tag="S")
                    nc.vector.tensor_tensor(
                        out=S[:], in0=iota_sb[:].unsqueeze(1).to_broadcast([P, T, P]),
                        in1=dloc.unsqueeze(2).to_broadcast([P, T, P]),
                        op=mybir.AluOpType.is_equal)

                    # St[d, e] one-hot (dst-major) via rank-1 PE broadcast
                    St = sb.tile([P, S128], BF16, tag="St")
                    for c0 in range(0, S128, 512):
                        c1 = min(c0 + 512, S128)
                        stb = ps.tile([P, 512], F32, tag="stb")
                        nc.tensor.matmul(out=stb[:, 0:c1 - c0],
                                         lhsT=ones1_sb[:],
                                         rhs=dlocT[:, c0:c1],
                                         start=True, stop=True)
                        nc.vector.tensor_scalar(
                            out=St[:, c0:c1], in0=stb[:, 0:c1 - c0],
                            scalar1=iotac_sb[:, 0:1], scalar2=None,
                            op0=mybir.AluOpType.is_equal)

                    # ad per edge: pad[e, h] = St[:,t]^T-matmul with adb
                    pad = ps1.tile([P, T * H1], F32, tag="pad")
                    for t in range(T):
                        nc.tensor.matmul(
                            out=pad[:, t * H1:(t + 1) * H1],
                            lhsT=St[:, t * P:(t + 1) * P],
                            rhs=ad1_sb[:, b, :],
                            start=True, stop=True)
                    padb = sm.tile([P, T * H1], BF16, tag="padb")
                    nc.scalar.copy(out=padb[:], in_=pad[:])

                    # exp(leaky(as + ad)) -> rhs[:, :, 0:H1]
                    sume = sm.tile([P, T * H1], BF16, tag="sume")
                    nc.vector.tensor_tensor(
                        out=sume[:, 0:TAb * H1].rearrange("p (t h) -> p t h", h=H1),
                        in0=heA[:, :, HID:U1],
                        in1=padb[:, 0:TAb * H1].rearrange("p (t h) -> p t h", h=H1),
                        op=mybir.AluOpType.add)
                    nc.vector.tensor_tensor(
                        out=sume[:, TAb * H1:].rearrange("p (t h) -> p t h", h=H1),
                        in0=heB[:, :, HID:U1],
                        in1=padb[:, TAb * H1:].rearrange("p (t h) -> p t h", h=H1),
                        op=mybir.AluOpType.add)
                    lk = sm.tile([P, T * H1], BF16, tag="lk")
                    nc.vector.scalar_tensor_tensor(
                        out=lk[:], in0=sume[:], scalar=LEAKY, in1=sume[:],
                        op0=mybir.AluOpType.mult, op1=mybir.AluOpType.max)
                    rhs = sb.tile([P, T, H1 + HID], BF16, tag="rhs")
                    nc.scalar.activation(
                        out=rhs[:, :, 0:H1],
                        in_=lk[:].rearrange("p (t h) -> p t h", h=H1),
                        func=mybir.ActivationFunctionType.Exp)
                    # Mw = h * ex (broadcast over the 32 chans of each head)
                    nc.vector.tensor_tensor(
                        out=rhs[:, 0:TAb, H1:].rearrange("p t (h c) -> p t h c", h=H1),
                        in0=heA[:, :, 0:HID].rearrange("p t (h c) -> p t h c", h=H1),
                        in1=rhs[:, 0:TAb, 0:H1].unsqueeze(3)
                        .to_broadcast([P, TAb, H1, C1]),
                        op=mybir.AluOpType.mult)
                    nc.vector.tensor_tensor(
                        out=rhs[:, TAb:T, H1:].rearrange("p t (h c) -> p t h c", h=H1),
                        in0=heB[:, :, 0:HID].rearrange("p t (h c) -> p t h c", h=H1),
                        in1=rhs[:, TAb:T, 0:H1].unsqueeze(3)
                        .to_broadcast([P, T - TAb, H1, C1]),
                        op=mybir.AluOpType.mult)

                    pm = ps.tile([P, H1 + HID], F32, tag="pm")
                    for t in range(T):
                        nc.tensor.matmul(out=pm[:], lhsT=S[:, t, :], rhs=rhs[:, t, :],
                                         start=(t == 0), stop=(t == T - 1))

                    # normalize + bias + ELU -> h2 block (f32)
                    srec = sm.tile([P, H1], F32, tag="srec")
                    nc.vector.tensor_scalar(
                        out=srec[:], in0=pm[:, 0:H1], scalar1=EPS, scalar2=None,
                        op0=mybir.AluOpType.add)
                    nc.vector.reciprocal(out=srec[:], in_=srec[:])
                    t2 = sm.tile([P, HID], F32, tag="t2")
                    nc.vector.tensor_tensor(
                        out=t2[:].rearrange("p (h c) -> p h c", h=H1),
                        in0=pm[:, H1:].rearrange("p (h c) -> p h c", h=H1),
                        in1=srec[:].unsqueeze(2).to_broadcast([P, H1, C1]),
                        op=mybir.AluOpType.mult)
                    nc.vector.tensor_tensor(out=t2[:], in0=t2[:], in1=b1_sb[:],
                                            op=mybir.AluOpType.add)
                    mm = sm.tile([P, HID], F32, tag="mm")
                    nc.vector.tensor_scalar(out=mm[:], in0=t2[:], scalar1=0.0,
                                            scalar2=None, op0=mybir.AluOpType.min)
                    qq = sm.tile([P, HID], F32, tag="qq")
                    nc.scalar.activation(out=qq[:], in_=mm[:],
                                         func=mybir.ActivationFunctionType.Exp)
                    pp = sm.tile([P, HID], F32, tag="pp")
                    nc.scalar.activation(out=pp[:], in_=t2[:],
                                         func=mybir.ActivationFunctionType.Relu)
                    h2 = sm.tile([P, HID], F32, tag="h2")
                    nc.vector.scalar_tensor_tensor(
                        out=h2[:], in0=qq[:], scalar=-1.0, in1=pp[:],
                        op0=mybir.AluOpType.add, op1=mybir.AluOpType.add)

                    # layer-2 dense for this block: g_ext = h2 @ W2F
                    h2T = sm.tile([P, KH, P], BF16, tag="h2T")
                    for k in range(KH):
                        ptr2 = ps1.tile([P, P], F32, tag="ptr")
                        nc.tensor.transpose(out=ptr2[:], in_=h2[:, k * P:(k + 1) * P],
                                            identity=ident_sb[:])
                        nc.scalar.copy(out=h2T[:, k, :], in_=ptr2[:])
                    pg = ps1.tile([P, U2 + 1], F32, tag="pg")
                    for k in range(KH):
                        nc.tensor.matmul(out=pg[:], lhsT=h2T[:, k, :],
                                         rhs=w2f_sb[:, k, :],
                                         start=(k == 0), stop=(k == KH - 1))
                    gr = sm.tile([P, U2], BF16, tag="gr")
                    nc.scalar.copy(out=gr[:], in_=pg[:, 0:U2])
                    if b < cfg.NB_LO:
                        s2dst = shard2_lo[b * P:(b + 1) * P, 0:U2]
                    else:
                        s2dst = shard2_hi[(b - cfg.NB_LO) * P:
                                          (b - cfg.NB_LO + 1) * P, 0:U2]
                    nc.scalar.dma_start(out=s2dst, in_=gr[:])
                    nc.scalar.copy(out=ad2_sb[:, b:b + 1], in_=pg[:, U2:U2 + 1])
            _stack.close()

            nc.gpsimd.collective_compute(
                "AllGather", mybir.AluOpType.bypass,
                replica_groups=[list(range(NCORES))],
                ins=[shard2_lo[:, :].opt()], outs=[table2_lo[:, :].opt()])
            nc.gpsimd.collective_compute(
                "AllGather", mybir.AluOpType.bypass,
                replica_groups=[list(range(NCORES))],
                ins=[shard2_hi[:, :].opt()], outs=[table2_hi[:, :].opt()])

            # ---------------- phase C: layer-2 edge aggregation
            with (
                tc.tile_pool(name="pc_ge", bufs=4) as p_ge,
                tc.tile_pool(name="pc_sb", bufs=2) as sb,
                tc.tile_pool(name="pc_small", bufs=3) as sm,
                tc.tile_pool(name="pc_ps", bufs=2, space="PSUM") as ps,
                tc.tile_pool(name="pc_ps1", bufs=1, space="PSUM") as ps1,
            ):
                icol = 0
                for b in range(NB):
                    T = TA2[b] + TB2[b]
                    S128 = T * P
                    tdl = sm.tile([P, 9 * T], I16, tag="tdl")
                    nc.sync.dma_start(out=tdl[:],
                                      in_=TDL2[:, 9 * icol: 9 * (icol + T)])
                    tidx = tdl[:, 0:8 * T]
                    dloc = tdl[:, 8 * T:9 * T].bitcast(BF16)
                    dlocT = sm.tile([1, S128], BF16, tag="dlocT")
                    nc.scalar.dma_start(
                        out=dlocT[:], in_=DLOCT2[:, icol * P: (icol + T) * P])

                    ge = p_ge.tile([P, T, TW2], BF16, tag="ge")
                    for c0, c1, tbl in _gather_chunks(
                            TA2[b], TB2[b], table2_lo[:, :], table2_hi[:, :]):
                        nc.gpsimd.dma_gather(
                            ge[:, c0:c1, :], tbl, tidx[:, 8 * c0:8 * c1],
                            num_idxs=(c1 - c0) * P, num_idxs_reg=(c1 - c0) * P,
                            elem_size=TW2)

                    S = sb.tile([P, T, P], BF16, tag="S")
                    nc.vector.tensor_tensor(
                        out=S[:], in0=iota_sb[:].unsqueeze(1).to_broadcast([P, T, P]),
                        in1=dloc.unsqueeze(2).to_broadcast([P, T, P]),
                        op=mybir.AluOpType.is_equal)

                    St = sb.tile([P, S128], BF16, tag="St")
                    for c0 in range(0, S128, 512):
                        c1 = min(c0 + 512, S128)
                        stb = ps.tile([P, 512], F32, tag="stb")
                        nc.tensor.matmul(out=stb[:, 0:c1 - c0],
                                         lhsT=ones1_sb[:],
                                         rhs=dlocT[:, c0:c1],
                                         start=True, stop=True)
                        nc.vector.tensor_scalar(
                            out=St[:, c0:c1], in0=stb[:, 0:c1 - c0],
                            scalar1=iotac_sb[:, 0:1], scalar2=None,
                            op0=mybir.AluOpType.is_equal)

                    adb2c = sm.tile([P, 1], BF16, tag="adb2c")
                    nc.scalar.copy(out=adb2c[:], in_=ad2_sb[:, b:b + 1])
                    pad2 = ps1.tile([P, T], F32, tag="pad")
                    for t in range(T):
                        nc.tensor.matmul(
                            out=pad2[:, t:t + 1],
                            lhsT=St[:, t * P:(t + 1) * P],
                            rhs=adb2c[:],
                            start=True, stop=True)
                    padb2 = sm.tile([P, T], BF16, tag="padb2")
                    nc.scalar.copy(out=padb2[:], in_=pad2[:])

                    sum2 = sm.tile([P, T], BF16, tag="sum2")
                    nc.vector.tensor_tensor(
                        out=sum2[:], in0=ge[:, :, OUT:U2].squeeze(2),
                        in1=padb2[:], op=mybir.AluOpType.add)
                    lk2 = sm.tile([P, T], BF16, tag="lk2")
                    nc.vector.scalar_tensor_tensor(
                        out=lk2[:], in0=sum2[:], scalar=LEAKY, in1=sum2[:],
                        op0=mybir.AluOpType.mult, op1=mybir.AluOpType.max)
                    rhs2 = sb.tile([P, T, 1 + OUT], BF16, tag="rhs2")
                    nc.scalar.activation(out=rhs2[:, :, 0:1],
                                         in_=lk2[:].unsqueeze(2),
                                         func=mybir.ActivationFunctionType.Exp)
                    nc.vector.tensor_tensor(
                        out=rhs2[:, :, 1:],
                        in0=ge[:, :, 0:OUT],
                        in1=rhs2[:, :, 0:1].to_broadcast([P, T, OUT]),
                        op=mybir.AluOpType.mult)

                    pm2 = ps.tile([P, 1 + OUT], F32, tag="pm")
                    for t in range(T):
                        nc.tensor.matmul(out=pm2[:], lhsT=S[:, t, :],
                                         rhs=rhs2[:, t, :],
                                         start=(t == 0), stop=(t == T - 1))

                    rec2 = sm.tile([P, 1], F32, tag="rec2")
                    nc.vector.tensor_scalar(
                        out=rec2[:], in0=pm2[:, 0:1], scalar1=EPS, scalar2=None,
                        op0=mybir.AluOpType.add)
                    nc.vector.reciprocal(out=rec2[:], in_=rec2[:])
                    ob = sm.tile([P, OUT], F32, tag="ob")
                    nc.vector.scalar_tensor_tensor(
                        out=ob[:], in0=pm2[:, 1:], scalar=rec2[:, 0:1], in1=b2_sb[:],
                        op0=mybir.AluOpType.mult, op1=mybir.AluOpType.add)
                    nrows = min(P, cfg.ND - b * P)
                    nc.scalar.dma_start(out=OUTT[b * P: b * P + nrows, :],
                                        in_=ob[0:nrows, :])
                    icol += T
    return nc


# ---------------------------------------------------------------- entry point
def gat_run(cfg, x, edge_index, W1, att_src1, att_dst1, b1, W2, att_src2,
            att_dst2, b2, trace=False):
    x = np.asarray(x, dtype=np.float32)
    edge_index = np.asarray(edge_index)
    W1f, W2f = make_weights(cfg, np.asarray(W1, np.float64),
                            np.asarray(att_src1, np.float64),
                            np.asarray(att_dst1, np.float64),
                            np.asarray(W2, np.float64),
                            np.asarray(att_src2, np.float64),
                            np.asarray(att_dst2, np.float64))
    ei = edge_index.astype(np.int64)
    idx16, dlocf, dloct, TA, TB = preprocess_graph(cfg, ei)
    Tsum = sum(TA) + sum(TB)
    idx16b, dlocfb, dloctb, TA2, TB2 = preprocess_graph(
        cfg, ei, row_of=_node_row2, half=cfg.HALF2)
    Tsum2 = sum(TA2) + sum(TB2)

    nc = build_kernel(cfg, TA, TB, Tsum, TA2, TB2, Tsum2)
    nc.finalize()

    P_, KI = P, cfg.IN_CH // P
    # reordered (core-major padded rows), transposed, per-block interleaved X
    xr = np.zeros((cfg.R, cfg.IN_CH), dtype=np.float32)
    for c in range(NCORES):
        xr[c * cfg.NPAD: c * cfg.NPAD + cfg.ND] = x[c * cfg.ND:(c + 1) * cfg.ND]
    # XTI[p, rb*KI*128 + k*128 + j] = xr[rb*128 + j, k*128 + p]
    xrT = np.ascontiguousarray(
        xr.reshape(cfg.RB, P_, KI, P_).transpose(3, 0, 2, 1))  # [p, rb, k, j]
    xti = bf16(xrT.reshape(P_, cfg.RB * KI * P_))

    w1f_r = bf16(W1f.reshape(KI, P_, cfg.U1 + cfg.H1).transpose(1, 0, 2))
    w2f_r = bf16(W2f.reshape(cfg.HID // P_, P_, cfg.U2 + 1).transpose(1, 0, 2))

    iota = bf16(np.broadcast_to(np.arange(P_, dtype=np.float32), (P_, P_)))
    iotac = np.arange(P_, dtype=np.float32)[:, None].copy()
    ones1 = bf16(np.ones((1, P_), dtype=np.float32))
    ident = np.eye(P_, dtype=np.float32)
    b1r = np.broadcast_to(np.asarray(b1, np.float32), (P_, cfg.HID)).copy()
    b2r = np.broadcast_to(np.asarray(b2, np.float32), (P_, cfg.OUT_CH)).copy()

    in_maps = []
    for c in range(NCORES):
        xsh = xr[c * cfg.NPAD:(c + 1) * cfg.NPAD]
        xshT = np.ascontiguousarray(
            xsh.reshape(cfg.NB, P_, KI, P_).transpose(3, 0, 2, 1))
        xsi = bf16(xshT.reshape(P_, cfg.NB * KI * P_))
        # merged tidx+dloc: per block segment, 8T idx cols then T dloc cols
        def mk_tdl(idxs, dlf, TAx, TBx, Tsumx):
            tdl = np.zeros((P_, 9 * Tsumx), dtype=np.int16)
            dloc_i16 = bf16(dlf).view(np.int16)
            col = 0
            icol = 0
            for b in range(len(TAx)):
                T = TAx[b] + TBx[b]
                tdl[:, col: col + 8 * T] = idxs[:, 8 * icol: 8 * (icol + T)]
                tdl[:, col + 8 * T: col + 9 * T] = dloc_i16[:, icol: icol + T]
                col += 9 * T
                icol += T
            return tdl
        in_maps.append({
            "XTI": xti, "XSI": xsi, "W1F": w1f_r, "W2F": w2f_r,
            "TDL": mk_tdl(idx16[c], dlocf[c], TA, TB, Tsum),
            "DLOCT": bf16(dloct[c]),
            "TDL2": mk_tdl(idx16b[c], dlocfb[c], TA2, TB2, Tsum2),
            "DLOCT2": bf16(dloctb[c]),
            "IOTA": iota, "IOTAC": iotac, "ONES1": ones1, "IDENT": ident,
            "B1R": b1r, "B2R": b2r,
        })
    res = run_bass_kernel_spmd(nc, in_maps, list(range(NCORES)), trace=trace)
    out = np.concatenate([res.results[c]["OUTT"] for c in range(NCORES)], axis=0)
    return out[:cfg.N], res


def kernel(x, edge_index, W1, att_src1, att_dst1, b1, W2, att_src2, att_dst2,
           b2):
    out, _ = gat_run(CFG_FULL, x, edge_index, W1, att_src1, att_dst1, b1, W2,
                     att_src2, att_dst2, b2)
    return out.astype(np.float32)



# revision 5
# speedup vs baseline: 1.6427x; 1.6427x over previous
"""GAT (2-layer, PyG-style) on 8 Trainium2 NeuronCores — v2.

Design vs baseline:
  - Layer-1 dense phase is REPLICATED on every core (PE is cheap), writing a
    local bf16 table [R, 384] = [h 256 | as 8 | pad], so the 64MB table-1
    AllGather disappears entirely.
  - All gather tables and PE matmuls in bf16 (2x PE rate, ~2x less HBM).
  - One dma_gather per (dst-block, half) instead of 6-tile chunks (amortizes
    the ~1us SWDGE fixed cost); descriptor ring enlarged.
  - One-hot "St" (dst-major) built via rank-1 PE broadcast of host-supplied
    dlocT + a single is_equal per 512-col chunk (no per-tile PE transposes).
  - X is fed host-pretransposed+interleaved so the dense phases do straight
    matmuls with no on-device transposes.
  - Node rows are laid out core-major (r = core*NPAD + local) for BOTH
    tables, so one int16 index set serves both layers.
"""
import sys
sys.path.insert(0, "/opt/trn_rl_repo")

import numpy as np
import concourse.bass as bass
import concourse.bacc as bacc
import concourse.mybir as mybir
from concourse.tile import TileContext
from concourse.bass_utils import run_bass_kernel_spmd

F32 = mybir.dt.float32
BF16 = mybir.dt.bfloat16
I16 = mybir.dt.int16

P = 128
NCORES = 8
LEAKY = 0.2
EPS = 1e-16


class Cfg:
    def __init__(self, N, E, IN_CH=256, HID=256, OUT_CH=64, H1=8):
        self.N, self.E = N, E
        self.IN_CH, self.HID, self.OUT_CH, self.H1 = IN_CH, HID, OUT_CH, H1
        self.C1 = HID // H1
        self.ND = N // NCORES                    # dst nodes per core
        self.NB = (self.ND + P - 1) // P         # dst blocks per core
        self.NPAD = self.NB * P                  # padded shard rows
        self.R = NCORES * self.NPAD              # global table rows
        self.RB = NCORES * self.NB               # global table blocks
        # asymmetric int16 split: smaller lo half -> fewer gather chunks
        # per block (TA~7 fits one 7-tile call) and lo table ready earlier
        self.HALF = 140 * P if self.RB == 392 else self.R // 2
        assert self.HALF % P == 0
        assert self.HALF < 32768 and (self.R - self.HALF) < 32768
        # layer-2 table split by shard-row halves so each half can be
        # AllGathered separately (first half fires mid-phase-B)
        self.NB_LO = 18 if self.NB == 49 else (self.NB + 1) // 2
        self.SL = self.NB_LO * P                 # shard rows in half 0
        self.HALF2 = NCORES * self.SL            # table2 half boundary
        assert self.HALF2 < 32768 and (self.R - self.HALF2) < 32768
        self.TW1 = 384                           # table1 row width bf16 (264 used)
        self.U1 = HID + H1                       # 264
        self.TW2 = 128                           # table2 row width bf16 (65 used)
        self.U2 = OUT_CH + 1                     # 65 = [g 64 | as2]


CFG_FULL = Cfg(N=50000, E=800000)


# ---------------------------------------------------------------- host side
def _node_row(cfg, n):
    # core-major padded row layout, same for table1 and table2
    return (n // cfg.ND) * cfg.NPAD + (n % cfg.ND)


def _node_row2(cfg, n):
    # layer-2 layout: shard-half-major so each half AllGathers contiguously
    c = n // cfg.ND
    l = n % cfg.ND
    return np.where(l < cfg.SL, c * cfg.SL + l,
                    cfg.HALF2 + c * (cfg.NPAD - cfg.SL) + (l - cfg.SL))


def preprocess_graph(cfg, edge_index, row_of=_node_row, half=None):
    """Per-core wrapped int16 gather indices, dloc (edge-major) and dlocT
    (flat, for the St broadcast), plus shared per-block tile counts."""
    if half is None:
        half = cfg.HALF
    src = np.concatenate([edge_index[0], np.arange(cfg.N, dtype=np.int64)])
    dst = np.concatenate([edge_index[1], np.arange(cfg.N, dtype=np.int64)])
    r_src = row_of(cfg, src)

    core = dst // cfg.ND
    dst_local = dst - core * cfg.ND
    blk = dst_local // P
    dloc = dst_local % P
    in_a = r_src < half

    NB = cfg.NB
    counts = np.zeros((NCORES, NB, 2), dtype=np.int64)
    np.add.at(counts, (core, blk, (~in_a).astype(np.int64)), 1)
    TA = np.maximum(1, (counts[:, :, 0].max(0) + P - 1) // P)
    TB = np.maximum(1, (counts[:, :, 1].max(0) + P - 1) // P)

    order = np.lexsort((in_a * -1, blk, core))  # by core, block, half (A first)
    rsrc_s, core_s, blk_s, dloc_s, ina_s = (
        r_src[order], core[order], blk[order], dloc[order], in_a[order])

    Tsum = int((TA + TB).sum())
    idx16 = []   # per core: [128, 8 * Tsum] int16
    dlocf = []   # per core: [128, Tsum] bf16-able float
    dloct = []   # per core: [1, Tsum*128] float
    for c in range(NCORES):
        iw = np.zeros((P, 8 * Tsum), dtype=np.int16)
        dw = np.full((P, Tsum), -1.0, dtype=np.float32)
        dt_ = np.full((1, Tsum * P), -1.0, dtype=np.float32)
        csel = core_s == c
        col0 = 0
        for b in range(NB):
            bsel = csel & (blk_s == b)
            for hh, T in ((0, int(TA[b])), (1, int(TB[b]))):
                hsel = bsel & (ina_s == (hh == 0))
                rr = rsrc_s[hsel] - (0 if hh == 0 else half)
                dd = dloc_s[hsel]
                S = T * P
                assert len(rr) <= S
                idx = np.zeros(S, dtype=np.int16)
                idx[: len(rr)] = rr.astype(np.int16)
                dl = np.full(S, -1.0, dtype=np.float32)
                dl[: len(dd)] = dd.astype(np.float32)
                w = idx.reshape(S // 16, 16).T          # [16, S/16] wrap
                iw[:, 8 * col0: 8 * col0 + S // 16] = np.tile(w, (8, 1))
                dw[:, col0: col0 + T] = dl.reshape(T, P).T
                dt_[0, col0 * P: col0 * P + S] = dl
                col0 += T
        assert col0 == Tsum
        idx16.append(iw)
        dlocf.append(dw)
        dloct.append(dt_)
    return idx16, dlocf, dloct, TA.astype(int).tolist(), TB.astype(int).tolist()


def make_weights(cfg, W1, att_src1, att_dst1, W2, att_src2, att_dst2):
    H1, C1 = cfg.H1, cfg.C1
    A1s = np.zeros((cfg.HID, H1), dtype=np.float64)
    A1s[np.arange(cfg.HID), np.arange(cfg.HID) // C1] = att_src1.ravel()
    A1d = np.zeros((cfg.HID, H1), dtype=np.float64)
    A1d[np.arange(cfg.HID), np.arange(cfg.HID) // C1] = att_dst1.ravel()
    # [h 256 | as 8 | ad 8] -> 272 cols
    W1f = np.concatenate([W1, W1 @ A1s, W1 @ A1d], axis=1).astype(np.float32)
    # [g 64 | as2 1 | ad2 1] -> 66 cols
    W2f = np.concatenate([W2, W2 @ att_src2.T, W2 @ att_dst2.T],
                         axis=1).astype(np.float32)
    return W1f, W2f


def bf16(a):
    import ml_dtypes
    return np.asarray(a, dtype=ml_dtypes.bfloat16)


# ---------------------------------------------------------------- device side
GATHER_CHUNK_TILES = 7   # 896 idxs per call; >=1280 fails on HW (ring limit)


def _gather_chunks(TA, TB, tbl_lo, tbl_hi):
    out = []
    for lo, hi, tbl in ((0, TA, tbl_lo), (TA, TA + TB, tbl_hi)):
        c = lo
        while c < hi:
            e = min(c + GATHER_CHUNK_TILES, hi)
            out.append((c, e, tbl))
            c = e
    return out


NQ = 4  # SWDGE queues; queue q runs desc-gen on Q7 cpu pair (2q, 2q+1)


def build_kernel(cfg, TA, TB, Tsum, TA2, TB2, Tsum2):
    nc = bacc.Bacc("TRN2", target_bir_lowering=False, debug=False,
                   num_devices=NCORES, num_swdge_queues=NQ)
    qctr = [0]

    def nextq():
        q = qctr[0] % NQ
        qctr[0] += 1
        return q
    IN, HID, OUT, H1, C1 = cfg.IN_CH, cfg.HID, cfg.OUT_CH, cfg.H1, cfg.C1
    U1, U2, TW1, TW2 = cfg.U1, cfg.U2, cfg.TW1, cfg.TW2
    NB, NPAD, R, RB, HALF = cfg.NB, cfg.NPAD, cfg.R, cfg.RB, cfg.HALF
    KI = IN // P   # k-chunks for layer-1 dense
    KH = HID // P  # k-chunks for layer-2 dense
    W1W = U1 + H1  # 272

    # host-pretransposed, per-block interleaved X: [128, RB*KI*128] bf16
    XTI = nc.declare_dram_parameter("XTI", [P, RB * KI * P], BF16, isOutput=False)
    XSI = nc.declare_dram_parameter("XSI", [P, NB * KI * P], BF16, isOutput=False)
    W1F = nc.declare_dram_parameter("W1F", [P, KI, W1W], BF16, isOutput=False)
    W2F = nc.declare_dram_parameter("W2F", [P, KH, U2 + 1], BF16, isOutput=False)
    TDL = nc.declare_dram_parameter("TDL", [P, 9 * Tsum], I16, isOutput=False)
    DLOCT = nc.declare_dram_parameter("DLOCT", [1, Tsum * P], BF16, isOutput=False)
    TDL2 = nc.declare_dram_parameter("TDL2", [P, 9 * Tsum2], I16, isOutput=False)
    DLOCT2 = nc.declare_dram_parameter("DLOCT2", [1, Tsum2 * P], BF16,
                                       isOutput=False)
    IOTA = nc.declare_dram_parameter("IOTA", [P, P], BF16, isOutput=False)
    IOTAC = nc.declare_dram_parameter("IOTAC", [P, 1], F32, isOutput=False)
    ONES1 = nc.declare_dram_parameter("ONES1", [1, P], BF16, isOutput=False)
    IDENT = nc.declare_dram_parameter("IDENT", [P, P], F32, isOutput=False)
    B1R = nc.declare_dram_parameter("B1R", [P, HID], F32, isOutput=False)
    B2R = nc.declare_dram_parameter("B2R", [P, OUT], F32, isOutput=False)
    OUTT = nc.declare_dram_parameter("OUTT", [cfg.ND, OUT], F32, isOutput=True)

    with TileContext(nc, num_cores=NCORES) as tc:
        with (
            tc.tile_pool(name="const", bufs=1) as cpool,
            tc.tile_pool(name="dram", bufs=1, space="DRAM") as dram,
        ):
            # resident constants
            iota_sb = cpool.tile([P, P], BF16)
            nc.sync.dma_start(out=iota_sb[:], in_=IOTA[:, :])
            iotac_sb = cpool.tile([P, 1], F32)
            nc.sync.dma_start(out=iotac_sb[:], in_=IOTAC[:, :])
            ones1_sb = cpool.tile([1, P], BF16)
            nc.sync.dma_start(out=ones1_sb[:], in_=ONES1[:, :])
            ident_sb = cpool.tile([P, P], F32)
            nc.sync.dma_start(out=ident_sb[:], in_=IDENT[:, :])
            b1_sb = cpool.tile([P, HID], F32)
            nc.sync.dma_start(out=b1_sb[:], in_=B1R[:, :])
            b2_sb = cpool.tile([P, OUT], F32)
            nc.sync.dma_start(out=b2_sb[:], in_=B2R[:, :])
            w1f_sb = cpool.tile([P, KI, W1W], BF16)
            for k in range(KI):
                nc.sync.dma_start(out=w1f_sb[:, k, :], in_=W1F[:, k, :])
            w2f_sb = cpool.tile([P, KH, U2 + 1], BF16)
            for k in range(KH):
                nc.sync.dma_start(out=w2f_sb[:, k, :], in_=W2F[:, k, :])
            # resident per-own-block attention-dst logits
            ad1_sb = cpool.tile([P, NB, H1], BF16)
            ad2_sb = cpool.tile([P, NB], F32)

            table1_lo = dram.tile([HALF, TW1], BF16)
            table1_hi = dram.tile([R - HALF, TW1], BF16)
            SL, HALF2 = cfg.SL, cfg.HALF2
            shard2_lo = dram.tile([SL, TW2], BF16)
            shard2_hi = dram.tile([NPAD - SL, TW2], BF16)
            table2_lo = dram.tile([HALF2, TW2], BF16, addr_space="Shared")
            table2_hi = dram.tile([R - HALF2, TW2], BF16, addr_space="Shared")

            # ---------------- phase A: replicated layer-1 dense, local table1
            # group size divides each table half so no group straddles lo/hi
            HB_LO = HALF // P
            GA = max(g for g in range(1, 9)
                     if HB_LO % g == 0 and (RB - HB_LO) % g == 0)
            NG_LO = HB_LO // GA
            # phase-B pools open FIRST so B's gather destinations do not
            # alias phase-A tiles (aliasing would serialize B behind A)
            KLA = min(5, NB)  # half-lo gather lookahead (blocks)
            pb_scope = (
                tc.tile_pool(name="pb_heA", bufs=KLA + 2),
                tc.tile_pool(name="pb_heB", bufs=2),
                tc.tile_pool(name="pb_tdl", bufs=KLA + 2),
                tc.tile_pool(name="pb_sb", bufs=2),
                tc.tile_pool(name="pb_small", bufs=2),
                tc.tile_pool(name="pb_dlt", bufs=2),
            )
            import contextlib as _cl
            _stack = _cl.ExitStack()
            p_heA = _stack.enter_context(pb_scope[0])
            p_heB = _stack.enter_context(pb_scope[1])
            p_tdl = _stack.enter_context(pb_scope[2])
            sb_b = _stack.enter_context(pb_scope[3])
            sm_b = _stack.enter_context(pb_scope[4])
            p_dlt = _stack.enter_context(pb_scope[5])
            with (
                tc.tile_pool(name="pa_sb", bufs=2) as sb,
                tc.tile_pool(name="pa_ps", bufs=4, space="PSUM") as ps,
            ):
                for g in range(RB // GA):
                    xt = sb.tile([P, GA, KI, P], BF16, tag="xt")
                    nc.sync.dma_start(
                        out=xt[:],
                        in_=XTI[:, g * GA * KI * P:(g + 1) * GA * KI * P])
                    hrow = sb.tile([P, GA, U1], BF16, tag="hrow")
                    for s in range(GA):
                        ph = ps.tile([P, U1], F32, tag="ph")
                        for k in range(KI):
                            nc.tensor.matmul(out=ph[:], lhsT=xt[:, s, k, :],
                                             rhs=w1f_sb[:, k, 0:U1],
                                             start=(k == 0), stop=(k == KI - 1))
                        if s % 2 == 0:
                            nc.scalar.copy(out=hrow[:, s, :], in_=ph[:])
                        else:
                            nc.vector.tensor_scalar(
                                out=hrow[:, s, :], in0=ph[:], scalar1=0.0,
                                scalar2=None, op0=mybir.AluOpType.add)
                    # [p, s, c] -> DRAM rows (g*GA+s)*128+p, cols 0:U1
                    if g < NG_LO:
                        wdst = table1_lo[g * GA * P:(g + 1) * GA * P, 0:U1]
                    else:
                        g2 = g - NG_LO
                        wdst = table1_hi[g2 * GA * P:(g2 + 1) * GA * P, 0:U1]
                    nc.scalar.dma_start(
                        out=wdst.rearrange("(s p) c -> p s c", p=P),
                        in_=hrow[:])

                # phase A': own-shard attention-dst logits (tiny, resident)
                GB = max(g for g in range(1, 9) if NB % g == 0)
                for g in range(NB // GB):
                    xs = sb.tile([P, GB, KI, P], BF16, tag="xs")
                    nc.sync.dma_start(
                        out=xs[:],
                        in_=XSI[:, g * GB * KI * P:(g + 1) * GB * KI * P])
                    for s in range(GB):
                        pa = ps.tile([P, H1], F32, tag="pa")
                        for k in range(KI):
                            nc.tensor.matmul(out=pa[:], lhsT=xs[:, s, k, :],
                                             rhs=w1f_sb[:, k, U1:W1W],
                                             start=(k == 0), stop=(k == KI - 1))
                        nc.scalar.copy(out=ad1_sb[:, g * GB + s, :], in_=pa[:])

            # ---------------- phase B: layer-1 edge aggregation + L2 dense
            # (pipelined: half-lo gathers issued KLA blocks ahead so their
            #  SWDGE descriptor generation overlaps phase A's second half)
            with (
                tc.tile_pool(name="pb_ps", bufs=2, space="PSUM") as ps,
                tc.tile_pool(name="pb_ps1", bufs=1, space="PSUM") as ps1,
            ):
                sb = sb_b
                sm = sm_b
                icols = []
                _ic = 0
                for b in range(NB):
                    icols.append(_ic)
                    _ic += TA[b] + TB[b]
                tdls = [None] * NB
                heAs = [None] * NB
                for i in range(NB + KLA):
                    if i < NB:
                        # issue stage: tdl load + half-lo gathers (depend only
                        # on table1_lo, ready at phase-A midpoint)
                        b = i
                        T = TA[b] + TB[b]
                        ic = icols[b]
                        tdl = p_tdl.tile([P, 9 * T], I16, tag="tdl")
                        nc.sync.dma_start(out=tdl[:],
                                          in_=TDL[:, 9 * ic: 9 * (ic + T)])
                        tdls[b] = tdl
                        heA = p_heA.tile([P, TA[b], TW1], BF16, tag="heA")
                        for c0 in range(0, TA[b], GATHER_CHUNK_TILES):
                            c1 = min(c0 + GATHER_CHUNK_TILES, TA[b])
                            nc.gpsimd.dma_gather(
                                heA[:, c0:c1, :], table1_lo[:, :],
                                tdl[:, 8 * c0:8 * c1],
                                num_idxs=(c1 - c0) * P,
                                num_idxs_reg=(c1 - c0) * P,
                                elem_size=TW1, queue_num=nextq())
                        heAs[b] = heA
                    if i < KLA:
                        continue
                    # compute stage (block b = i - KLA): half-hi gather + rest
                    b = i - KLA
                    T = TA[b] + TB[b]
                    TAb = TA[b]
                    S128 = T * P
                    ic = icols[b]
                    tdl = tdls[b]
                    heA = heAs[b]
                    tdls[b] = heAs[b] = None
                    dloc = tdl[:, 8 * T:9 * T].bitcast(BF16)

                    heB = p_heB.tile([P, TB[b], TW1], BF16, tag="heB")
                    for c0 in range(0, TB[b], GATHER_CHUNK_TILES):
                        c1 = min(c0 + GATHER_CHUNK_TILES, TB[b])
                        nc.gpsimd.dma_gather(
                            heB[:, c0:c1, :], table1_hi[:, :],
                            tdl[:, 8 * (TAb + c0):8 * (TAb + c1)],
                            num_idxs=(c1 - c0) * P,
                            num_idxs_reg=(c1 - c0) * P,
                            elem_size=TW1, queue_num=nextq())

                    dlocT = p_dlt.tile([1, S128], BF16, tag="dlocT")
                    nc.scalar.dma_start(
                        out=dlocT[:], in_=DLOCT[:, ic * P: (ic + T) * P])

                    # S[e, t, d] one-hot (edge-major) for aggregation
                    S = sb.tile([P, T, P], BF16, tag="S")
                    nc.vector.tensor_tensor(
                        out=S[:], in0=iota_sb[:].unsqueeze(1).to_broadcast([P, T, P]),
                        in1=dloc.unsqueeze(2).to_broadcast([P, T, P]),
                        op=mybir.AluOpType.is_equal)

                    # St[d, e] one-hot (dst-major) via rank-1 PE broadcast
                    St = sb.tile([P, S128], BF16, tag="St")
                    for c0 in range(0, S128, 512):
                        c1 = min(c0 + 512, S128)
                        stb = ps.tile([P, 512], F32, tag="stb")
                        nc.tensor.matmul(out=stb[:, 0:c1 - c0],
                                         lhsT=ones1_sb[:],
                                         rhs=dlocT[:, c0:c1],
                                         start=True, stop=True)
                        nc.vector.tensor_scalar(
                            out=St[:, c0:c1], in0=stb[:, 0:c1 - c0],
                            scalar1=iotac_sb[:, 0:1], scalar2=None,
                            op0=mybir.AluOpType.is_equal)

                    # ad per edge: pad[e, h] = St[:,t]^T-matmul with adb
                    pad = ps1.tile([P, T * H1], F32, tag="pad")
                    for t in range(T):
                        nc.tensor.matmul(
                            out=pad[:, t * H1:(t + 1) * H1],
                            lhsT=St[:, t * P:(t + 1) * P],
                            rhs=ad1_sb[:, b, :],
                            start=True, stop=True)
                    padb = sm.tile([P, T * H1], BF16, tag="padb")
                    nc.scalar.copy(out=padb[:], in_=pad[:])

                    # exp(leaky(as + ad)) -> rhs[:, :, 0:H1]
                    sume = sm.tile([P, T * H1], BF16, tag="sume")
                    nc.vector.tensor_tensor(
                        out=sume[:, 0:TAb * H1].rearrange("p (t h) -> p t h", h=H1),
                        in0=heA[:, :, HID:U1],
                        in1=padb[:, 0:TAb * H1].rearrange("p (t h) -> p t h", h=H1),
                        op=mybir.AluOpType.add)
                    nc.vector.tensor_tensor(
                        out=sume[:, TAb * H1:].rearrange("p (t h) -> p t h", h=H1),
                        in0=heB[:, :, HID:U1],
                        in1=padb[:, TAb * H1:].rearrange("p (t h) -> p t h", h=H1),
                        op=mybir.AluOpType.add)
                    lk = sm.tile([P, T * H1], BF16, tag="lk")
                    nc.vector.scalar_tensor_tensor(
                        out=lk[:], in0=sume[:], scalar=LEAKY, in1=sume[:],
                        op0=mybir.AluOpType.mult, op1=mybir.AluOpType.max)
                    rhs = sb.tile([P, T, H1 + HID], BF16, tag="rhs")
                    nc.scalar.activation(
                        out=rhs[:, :, 0:H1],
                        in_=lk[:].rearrange("p (t h) -> p t h", h=H1),
                        func=mybir.ActivationFunctionType.Exp)
                    # Mw = h * ex (broadcast over the 32 chans of each head)
                    nc.vector.tensor_tensor(
                        out=rhs[:, 0:TAb, H1:].rearrange("p t (h c) -> p t h c", h=H1),
                        in0=heA[:, :, 0:HID].rearrange("p t (h c) -> p t h c", h=H1),
                        in1=rhs[:, 0:TAb, 0:H1].unsqueeze(3)
                        .to_broadcast([P, TAb, H1, C1]),
                        op=mybir.AluOpType.mult)
                    nc.vector.tensor_tensor(
                        out=rhs[:, TAb:T, H1:].rearrange("p t (h c) -> p t h c", h=H1),
                        in0=heB[:, :, 0:HID].rearrange("p t (h c) -> p t h c", h=H1),
                        in1=rhs[:, TAb:T, 0:H1].unsqueeze(3)
                        .to_broadcast([P, T - TAb, H1, C1]),
                        op=mybir.AluOpType.mult)

                    pm = ps.tile([P, H1 + HID], F32, tag="pm")
                    for t in range(T):
                        nc.tensor.matmul(out=pm[:], lhsT=S[:, t, :], rhs=rhs[:, t, :],
                                         start=(t == 0), stop=(t == T - 1))

                    # normalize + bias + ELU -> h2 block (f32)
                    srec = sm.tile([P, H1], F32, tag="srec")
                    nc.vector.tensor_scalar(
                        out=srec[:], in0=pm[:, 0:H1], scalar1=EPS, scalar2=None,
                        op0=mybir.AluOpType.add)
                    nc.vector.reciprocal(out=srec[:], in_=srec[:])
                    t2 = sm.tile([P, HID], F32, tag="t2")
                    nc.vector.tensor_tensor(
                        out=t2[:].rearrange("p (h c) -> p h c", h=H1),
                        in0=pm[:, H1:].rearrange("p (h c) -> p h c", h=H1),
                        in1=srec[:].unsqueeze(2).to_broadcast([P, H1, C1]),
                        op=mybir.AluOpType.mult)
                    nc.vector.tensor_tensor(out=t2[:], in0=t2[:], in1=b1_sb[:],
                                            op=mybir.AluOpType.add)
                    mm = sm.tile([P, HID], F32, tag="mm")
                    nc.vector.tensor_scalar(out=mm[:], in0=t2[:], scalar1=0.0,
                                            scalar2=None, op0=mybir.AluOpType.min)
                    qq = sm.tile([P, HID], F32, tag="qq")
                    nc.scalar.activation(out=qq[:], in_=mm[:],
                                         func=mybir.ActivationFunctionType.Exp)
                    pp = sm.tile([P, HID], F32, tag="pp")
                    nc.scalar.activation(out=pp[:], in_=t2[:],
                                         func=mybir.ActivationFunctionType.Relu)
                    h2 = sm.tile([P, HID], F32, tag="h2")
                    nc.vector.scalar_tensor_tensor(
                        out=h2[:], in0=qq[:], scalar=-1.0, in1=pp[:],
                        op0=mybir.AluOpType.add, op1=mybir.AluOpType.add)

                    # layer-2 dense for this block: g_ext = h2 @ W2F
                    h2T = sm.tile([P, KH, P], BF16, tag="h2T")
                    for k in range(KH):
                        ptr2 = ps1.tile([P, P], F32, tag="ptr")
                        nc.tensor.transpose(out=ptr2[:], in_=h2[:, k * P:(k + 1) * P],
                                            identity=ident_sb[:])
                        nc.scalar.copy(out=h2T[:, k, :], in_=ptr2[:])
                    pg = ps1.tile([P, U2 + 1], F32, tag="pg")
                    for k in range(KH):
                        nc.tensor.matmul(out=pg[:], lhsT=h2T[:, k, :],
                                         rhs=w2f_sb[:, k, :],
                                         start=(k == 0), stop=(k == KH - 1))
                    gr = sm.tile([P, U2], BF16, tag="gr")
                    nc.scalar.copy(out=gr[:], in_=pg[:, 0:U2])
                    if b < cfg.NB_LO:
                        s2dst = shard2_lo[b * P:(b + 1) * P, 0:U2]
                    else:
                        s2dst = shard2_hi[(b - cfg.NB_LO) * P:
                                          (b - cfg.NB_LO + 1) * P, 0:U2]
                    nc.scalar.dma_start(out=s2dst, in_=gr[:])
                    nc.scalar.copy(out=ad2_sb[:, b:b + 1], in_=pg[:, U2:U2 + 1])
            _stack.close()

            nc.gpsimd.collective_compute(
                "AllGather", mybir.AluOpType.bypass,
                replica_groups=[list(range(NCORES))],
                ins=[shard2_lo[:, :].opt()], outs=[table2_lo[:, :].opt()])
            nc.gpsimd.collective_compute(
                "AllGather", mybir.AluOpType.bypass,
                replica_groups=[list(range(NCORES))],
                ins=[shard2_hi[:, :].opt()], outs=[table2_hi[:, :].opt()])

            # ---------------- phase C: layer-2 edge aggregation
            with (
                tc.tile_pool(name="pc_ge", bufs=4) as p_ge,
                tc.tile_pool(name="pc_sb", bufs=2) as sb,
                tc.tile_pool(name="pc_small", bufs=3) as sm,
                tc.tile_pool(name="pc_ps", bufs=2, space="PSUM") as ps,
                tc.tile_pool(name="pc_ps1", bufs=1, space="PSUM") as ps1,
            ):
                icol = 0
                for b in range(NB):
                    T = TA2[b] + TB2[b]
                    S128 = T * P
                    tdl = sm.tile([P, 9 * T], I16, tag="tdl")
                    nc.sync.dma_start(out=tdl[:],
                                      in_=TDL2[:, 9 * icol: 9 * (icol + T)])
                    tidx = tdl[:, 0:8 * T]
                    dloc = tdl[:, 8 * T:9 * T].bitcast(BF16)
                    dlocT = sm.tile([1, S128], BF16, tag="dlocT")
                    nc.scalar.dma_start(
                        out=dlocT[:], in_=DLOCT2[:, icol * P: (icol + T) * P])

                    ge = p_ge.tile([P, T, TW2], BF16, tag="ge")
                    for c0, c1, tbl in _gather_chunks(
                            TA2[b], TB2[b], table2_lo[:, :], table2_hi[:, :]):
                        nc.gpsimd.dma_gather(
                            ge[:, c0:c1, :], tbl, tidx[:, 8 * c0:8 * c1],
                            num_idxs=(c1 - c0) * P, num_idxs_reg=(c1 - c0) * P,
                            elem_size=TW2, queue_num=nextq())

                    S = sb.tile([P, T, P], BF16, tag="S")
                    nc.vector.tensor_tensor(
                        out=S[:], in0=iota_sb[:].unsqueeze(1).to_broadcast([P, T, P]),
                        in1=dloc.unsqueeze(2).to_broadcast([P, T, P]),
                        op=mybir.AluOpType.is_equal)

                    St = sb.tile([P, S128], BF16, tag="St")
                    for c0 in range(0, S128, 512):
                        c1 = min(c0 + 512, S128)
                        stb = ps.tile([P, 512], F32, tag="stb")
                        nc.tensor.matmul(out=stb[:, 0:c1 - c0],
                                         lhsT=ones1_sb[:],
                                         rhs=dlocT[:, c0:c1],
                                         start=True, stop=True)
                        nc.vector.tensor_scalar(
                            out=St[:, c0:c1], in0=stb[:, 0:c1 - c0],
                            scalar1=iotac_sb[:, 0:1], scalar2=None,
                            op0=mybir.AluOpType.is_equal)

                    adb2c = sm.tile([P, 1], BF16, tag="adb2c")
                    nc.scalar.copy(out=adb2c[:], in_=ad2_sb[:, b:b + 1])
                    pad2 = ps1.tile([P, T], F32, tag="pad")
                    for t in range(T):
                        nc.tensor.matmul(
                            out=pad2[:, t:t + 1],
                            lhsT=St[:, t * P:(t + 1) * P],
                            rhs=adb2c[:],
                            start=True, stop=True)
                    padb2 = sm.tile([P, T], BF16, tag="padb2")
                    nc.scalar.copy(out=padb2[:], in_=pad2[:])

                    sum2 = sm.tile([P, T], BF16, tag="sum2")
                    nc.vector.tensor_tensor(
                        out=sum2[:], in0=ge[:, :, OUT:U2].squeeze(2),
                        in1=padb2[:], op=mybir.AluOpType.add)
                    lk2 = sm.tile([P, T], BF16, tag="lk2")
                    nc.vector.scalar_tensor_tensor(
                        out=lk2[:], in0=sum2[:], scalar=LEAKY, in1=sum2[:],
                        op0=mybir.AluOpType.mult, op1=mybir.AluOpType.max)
                    rhs2 = sb.tile([P, T, 1 + OUT], BF16, tag="rhs2")
                    nc.scalar.activation(out=rhs2[:, :, 0:1],
                                         in_=lk2[:].unsqueeze(2),
                                         func=mybir.ActivationFunctionType.Exp)
                    nc.vector.tensor_tensor(
                        out=rhs2[:, :, 1:],
                        in0=ge[:, :, 0:OUT],
                        in1=rhs2[:, :, 0:1].to_broadcast([P, T, OUT]),
                        op=mybir.AluOpType.mult)

                    pm2 = ps.tile([P, 1 + OUT], F32, tag="pm")
                    for t in range(T):
                        nc.tensor.matmul(out=pm2[:], lhsT=S[:, t, :],
                                         rhs=rhs2[:, t, :],
                                         start=(t == 0), stop=(t == T - 1))

                    rec2 = sm.tile([P, 1], F32, tag="rec2")
                    nc.vector.tensor_scalar(
                        out=rec2[:], in0=pm2[:, 0:1], scalar1=EPS, scalar2=None,
                        op0=mybir.AluOpType.add)
                    nc.vector.reciprocal(out=rec2[:], in_=rec2[:])
                    ob = sm.tile([P, OUT], F32, tag="ob")
                    nc.vector.scalar_tensor_tensor(
                        out=ob[:], in0=pm2[:, 1:], scalar=rec2[:, 0:1], in1=b2_sb[:],
                        op0=mybir.AluOpType.mult, op1=mybir.AluOpType.add)
                    nrows = min(P, cfg.ND - b * P)
                    nc.scalar.dma_start(out=OUTT[b * P: b * P + nrows, :],
                                        in_=ob[0:nrows, :])
                    icol += T
    return nc


# ---------------------------------------------------------------- entry point
def gat_run(cfg, x, edge_index, W1, att_src1, att_dst1, b1, W2, att_src2,
            att_dst2, b2, trace=False):
    x = np.asarray(x, dtype=np.float32)
    edge_index = np.asarray(edge_index)
    W1f, W2f = make_weights(cfg, np.asarray(W1, np.float64),
                            np.asarray(att_src1, np.float64),
                            np.asarray(att_dst1, np.float64),
                            np.asarray(W2, np.float64),
                            np.asarray(att_src2, np.float64),
                            np.asarray(att_dst2, np.float64))
    ei = edge_index.astype(np.int64)
    idx16, dlocf, dloct, TA, TB = preprocess_graph(cfg, ei)
    Tsum = sum(TA) + sum(TB)
    idx16b, dlocfb, dloctb, TA2, TB2 = preprocess_graph(
        cfg, ei, row_of=_node_row2, half=cfg.HALF2)
    Tsum2 = sum(TA2) + sum(TB2)

    nc = build_kernel(cfg, TA, TB, Tsum, TA2, TB2, Tsum2)
    nc.finalize()

    P_, KI = P, cfg.IN_CH // P
    # reordered (core-major padded rows), transposed, per-block interleaved X
    xr = np.zeros((cfg.R, cfg.IN_CH), dtype=np.float32)
    for c in range(NCORES):
        xr[c * cfg.NPAD: c * cfg.NPAD + cfg.ND] = x[c * cfg.ND:(c + 1) * cfg.ND]
    # XTI[p, rb*KI*128 + k*128 + j] = xr[rb*128 + j, k*128 + p]
    xrT = np.ascontiguousarray(
        xr.reshape(cfg.RB, P_, KI, P_).transpose(3, 0, 2, 1))  # [p, rb, k, j]
    xti = bf16(xrT.reshape(P_, cfg.RB * KI * P_))

    w1f_r = bf16(W1f.reshape(KI, P_, cfg.U1 + cfg.H1).transpose(1, 0, 2))
    w2f_r = bf16(W2f.reshape(cfg.HID // P_, P_, cfg.U2 + 1).transpose(1, 0, 2))

    iota = bf16(np.broadcast_to(np.arange(P_, dtype=np.float32), (P_, P_)))
    iotac = np.arange(P_, dtype=np.float32)[:, None].copy()
    ones1 = bf16(np.ones((1, P_), dtype=np.float32))
    ident = np.eye(P_, dtype=np.float32)
    b1r = np.broadcast_to(np.asarray(b1, np.float32), (P_, cfg.HID)).copy()
    b2r = np.broadcast_to(np.asarray(b2, np.float32), (P_, cfg.OUT_CH)).copy()

    in_maps = []
    for c in range(NCORES):
        xsh = xr[c * cfg.NPAD:(c + 1) * cfg.NPAD]
        xshT = np.ascontiguousarray(
            xsh.reshape(cfg.NB, P_, KI, P_).transpose(3, 0, 2, 1))
        xsi = bf16(xshT.reshape(P_, cfg.NB * KI * P_))
        # merged tidx+dloc: per block segment, 8T idx cols then T dloc cols
        def mk_tdl(idxs, dlf, TAx, TBx, Tsumx):
            tdl = np.zeros((P_, 9 * Tsumx), dtype=np.int16)
            dloc_i16 = bf16(dlf).view(np.int16)
            col = 0
            icol = 0
            for b in range(len(TAx)):
                T = TAx[b] + TBx[b]
                tdl[:, col: col + 8 * T] = idxs[:, 8 * icol: 8 * (icol + T)]
                tdl[:, col + 8 * T: col + 9 * T] = dloc_i16[:, icol: icol + T]
                col += 9 * T
                icol += T
            return tdl
        in_maps.append({
            "XTI": xti, "XSI": xsi, "W1F": w1f_r, "W2F": w2f_r,
            "TDL": mk_tdl(idx16[c], dlocf[c], TA, TB, Tsum),
            "DLOCT": bf16(dloct[c]),
            "TDL2": mk_tdl(idx16b[c], dlocfb[c], TA2, TB2, Tsum2),
            "DLOCT2": bf16(dloctb[c]),
            "IOTA": iota, "IOTAC": iotac, "ONES1": ones1, "IDENT": ident,
            "B1R": b1r, "B2R": b2r,
        })
    res = run_bass_kernel_spmd(nc, in_maps, list(range(NCORES)), trace=trace)
    out = np.concatenate([res.results[c]["OUTT"] for c in range(NCORES)], axis=0)
    return out[:cfg.N], res


def kernel(x, edge_index, W1, att_src1, att_dst1, b1, W2, att_src2, att_dst2,
           b2):
    out, _ = gat_run(CFG_FULL, x, edge_index, W1, att_src1, att_dst1, b1, W2,
                     att_src2, att_dst2, b2)
    return out.astype(np.float32)

